# revision 40
# baseline (speedup 1.0000x reference)
"""Trainium2 Bass kernel for EfficientAttention (linear attention block).

Computation (per batch b, head h):
    qkv = x @ w_qkv.T + b_qkv
    q = softmax(q, axis=head_dim) * head_dim**-0.5
    k = softmax(k, axis=seqlen)
    kv[d,e] = sum_s k[s,d] v[s,e]          (per-head 64x64 state)
    out[s,e] = sum_d q[s,d] kv[d,e]
    y = out @ w_proj.T + b_proj

Sharding: 8 cores = (batch b = c//2, seq half = c%2); 2048 tokens per core,
all 16 heads. The only cross-core coupling is the kv state and the
k-softmax denominator Z (sums over the full 4096 seqlen) -> one small
AllReduce (pairs of cores) of [128, 520] fp32.

v2 design (vs the fp32r baseline):
- All matmul operands in bf16 (1 cycle/row on the PE vs ~2.2 measured for
  fp32 mode; fast weight loads; half the DMA bytes). PSUM stays fp32.
- q projection computed directly d-major (weights stationary, x moving):
  no PE transposes, no per-head ACT normalize sweep. The q-softmax
  denominator comes from one extra matmul per block against a
  block-diagonal 8s mask (folds the 1/sqrt(HD) scale), reciprocal on DVE,
  one elementwise multiply.
- kv state + k-softmax Z accumulate in PSUM banks across the whole token
  loop (per-element has_written accumulation); no DVE adds, and Z lands
  directly in the [128, 8] layout the collective stage wants.
- PE warmup matmuls during the initial weight-DMA wait.
"""

import sys

sys.path.insert(0, "/opt/trn_rl_repo")

import numpy as np

import concourse.bacc as bacc
import concourse.tile as tile
from concourse import mybir
from concourse import bass_utils

F32 = mybir.dt.float32
BF16 = mybir.dt.bfloat16

D = 1024          # model dim (= qkv contraction dim)
T = 2048          # tokens per core (one batch element's half sequence)
NH = 16           # heads
HD = 64           # head dim
NPAIR = 8         # head pairs (2 heads / 128 partitions)
KC = D // 128     # contraction chunks of 128
TB = T // 128     # token blocks of 128
SB = T // 512     # token column blocks of 512
SCALE = HD ** -0.5

N_CORES = 8


def build_program(with_bias=False):
    nc = bacc.Bacc("TRN2", target_bir_lowering=False, num_devices=N_CORES)

    xt = nc.dram_tensor("xt", [D, T], BF16, kind="ExternalInput")      # x chunk, transposed
    xblk = nc.dram_tensor("xblk", [TB, 128, KC, 128], BF16,
                          kind="ExternalInput")  # phase-1 tiles, contiguous
    wq = nc.dram_tensor("wq", [D, D], BF16, kind="ExternalInput")      # w_q.T
    wk = nc.dram_tensor("wk", [D, D], BF16, kind="ExternalInput")      # w_k.T
    wv = nc.dram_tensor("wv", [D, D], BF16, kind="ExternalInput")      # w_v.T
    wp = nc.dram_tensor("wp", [D, D], BF16, kind="ExternalInput")      # w_proj.T
    bq = nc.dram_tensor("bq", [D], F32, kind="ExternalInput")
    bk = nc.dram_tensor("bk", [D], F32, kind="ExternalInput")
    bv = nc.dram_tensor("bv", [D], F32, kind="ExternalInput")
    bp = nc.dram_tensor("bp", [D], F32, kind="ExternalInput")
    cst = nc.dram_tensor("cst", [128, 132], BF16, kind="ExternalInput")  # M8 | ones | pad
    y = nc.dram_tensor("y", [T, D], F32, kind="ExternalOutput")

    xt_v = xt.rearrange("(kc p) t -> p kc t", p=128)
    wq_v = wq.rearrange("(kc p) f -> p kc f", p=128)
    wk_v = wk.rearrange("(kc p) f -> p kc f", p=128)
    wv_v = wv.rearrange("(kc p) f -> p kc f", p=128)
    wp_v = wp.rearrange("(kc p) f -> p kc f", p=128)

    def bias_bcast(b):
        # DRAM [D] broadcast-load to SBUF [128, D] (partition step 0)
        import concourse.bass as bass
        ap = b[:]
        return bass.AP(tensor=ap.tensor, offset=ap.offset, ap=[[0, 128]] + list(ap.ap))

    with tile.TileContext(nc) as tc:
        with (
            tc.tile_pool(name="const", bufs=1) as const,
            tc.tile_pool(name="wpool", bufs=1) as wpool,
            tc.tile_pool(name="xin", bufs=3) as xin,
            tc.tile_pool(name="xq", bufs=2) as xqp,
            tc.tile_pool(name="ekv", bufs=2) as ekv,
            tc.tile_pool(name="acc", bufs=1) as accp,
            tc.tile_pool(name="qpool", bufs=2) as qpool,
            tc.tile_pool(name="qt", bufs=1) as qtpool,
            tc.tile_pool(name="atn", bufs=2) as atnp,
            tc.tile_pool(name="kvsb", bufs=1) as kvsbp,
            tc.tile_pool(name="yout", bufs=3) as youtp,
            tc.tile_pool(name="psum", bufs=2, space="PSUM") as psum,
            tc.tile_pool(name="dram", bufs=1, space="DRAM") as dram,
        ):
            cst_sb = const.tile([128, 132], BF16, tag="cst")
            nc.sync.dma_start(cst_sb, cst[:])
            m8 = cst_sb[:, 0:128]
            ones = cst_sb[:, 128:129]

            if with_bias:
                bk_sb = const.tile([128, D], BF16, tag="bk")
                bv_sb = const.tile([128, D], BF16, tag="bv")
                bp_sb = const.tile([128, D], BF16, tag="bp")
                nc.gpsimd.dma_start(bk_sb, bias_bcast(bk))
                nc.gpsimd.dma_start(bv_sb, bias_bcast(bv))
                nc.gpsimd.dma_start(bp_sb, bias_bcast(bp))
                bq_col = const.tile([128, KC], F32, tag="bqc")
                nc.gpsimd.dma_start(bq_col, bq[:].rearrange("(kc p) -> p kc", p=128))

            # Weights chunked (256KB each) so the k projection can start as
            # soon as early chunks land: wk on the gpsimd ring; wv on the
            # sync ring right behind the first two x tiles; wq/wp trickle
            # in behind wk with phase-1 of slack.
            wkbig = wpool.tile([128, KC, D], BF16, tag="wk", name="wk")
            wvbig = wpool.tile([128, KC, D], BF16, tag="wv", name="wv")
            wqbig = wpool.tile([128, KC, D], BF16, tag="wq", name="wq")
            wpbig = wpool.tile([128, KC, D], BF16, tag="wp", name="wp")
            wk_sb = [wkbig[:, kc, :] for kc in range(KC)]
            wv_sb = [wvbig[:, kc, :] for kc in range(KC)]
            wq_sb = [wqbig[:, kc, :] for kc in range(KC)]
            wp_sb = [wpbig[:, kc, :] for kc in range(KC)]
            NSTART = 4  # tbs processed weight-chunk-major at startup
            # The DMA rings are latency/ramp-bound when cold (a 256KB chunk
            # at the ring head completes ~9.7us; a 33KB one ~2.8us), so the
            # first two wk chunks go out as 64KB quarter-pieces on the sync
            # ring — the kc0/kc1 matmuls depend region-wise on just the
            # pieces they read and start several us earlier.
            for kc in range(2):
                for q4 in range(4):
                    nc.sync.dma_start(wk_sb[kc][:, q4 * 256:(q4 + 1) * 256],
                                      wk_v[:, kc, q4 * 256:(q4 + 1) * 256])
            for kc in range(2, KC):
                nc.gpsimd.dma_start(wk_sb[kc], wk_v[:, kc, :])
            xpre = []
            for tb in range(NSTART):
                xtile = xin.tile([128, KC, 128], BF16, tag="x", bufs=6)
                nc.sync.dma_start(xtile, xblk[tb, :, :, :])
                xpre.append(xtile)
            for kc in range(KC):
                nc.sync.dma_start(wv_sb[kc], wv_v[:, kc, :])
            for kc in range(KC):
                nc.gpsimd.dma_start(wq_sb[kc], wq_v[:, kc, :])
                nc.gpsimd.dma_start(wp_sb[kc], wp_v[:, kc, :])

            # PE warmup: dummy matmuls on the (tiny, early-arriving) const
            # tile keep the HAM activity window busy while weights stream in,
            # so real matmuls start at the full 2.4 GHz clock.
            for w in range(24):
                wps = psum.tile([128, 512], F32, tag="a")
                nc.tensor.matmul(wps[:, 0:128], m8, m8, start=True, stop=True)

            # ---- Phase 1 startup: tbs 0..3 processed weight-chunk-major.
            # k projections for 4 tbs x 2 halves fill all 8 PSUM banks, so
            # every arriving wk chunk immediately feeds 8 matmuls (~1.7us of
            # PE work per ~1.4us DMA): the PE streams through the weight
            # window instead of stalling until the full matrix lands.
            tag8 = ("a", "a", "b", "b", "c", "d", "e", "f")

            def startup_proj(w_sb, nm):
                tiles = [psum.tile([128, 512], F32, tag=tag8[i],
                                   bufs=(2 if tag8[i] in ("a", "b") else 1),
                                   name=f"sp_{nm}{i}")
                         for i in range(8)]
                for kc in range(KC):
                    for i in range(8):
                        t, half = divmod(i, 2)
                        nc.tensor.matmul(
                            tiles[i], xpre[t][:, kc, :],
                            w_sb[kc][:, half * 512:(half + 1) * 512],
                            start=(kc == 0), stop=(kc == KC - 1))
                return tiles

            ek_s = [ekv.tile([128, D], BF16, tag="ek", bufs=5, name=f"eks{t}")
                    for t in range(NSTART)]
            vv_s = [ekv.tile([128, D], BF16, tag="v", bufs=5, name=f"vvs{t}")
                    for t in range(NSTART)]
            ktiles = startup_proj(wk_sb, "k")
            for i in range(8):
                t, half = divmod(i, 2)
                sl = slice(half * 512, (half + 1) * 512)
                if with_bias:
                    nc.vector.tensor_add(ktiles[i], ktiles[i], bk_sb[:, sl])
                nc.scalar.activation(ek_s[t][:, sl], ktiles[i],
                                     mybir.ActivationFunctionType.Exp)
            vtiles = startup_proj(wv_sb, "v")
            for i in range(8):
                t, half = divmod(i, 2)
                sl = slice(half * 512, (half + 1) * 512)
                if with_bias:
                    nc.vector.tensor_add(vtiles[i], vtiles[i], bv_sb[:, sl])
                nc.scalar.copy(vv_s[t][:, sl], vtiles[i])

            # kv pairs: bankA = pairs 0-3 (cols 128*p), bankB = pairs 4-7.
            # zk bank: col p = Z partial for pair p's dims [128, 8].
            # start=True only on the first write of each bank (clears the
            # whole bank; the other pairs' first writes land on cleared
            # has_written bits and overwrite).
            kvps = [psum.tile([128, 512], F32, tag=t, name=f"kv{t}", bufs=1)
                    for t in ("c", "d")]
            zkps = psum.tile([128, 512], F32, tag="e", name="zkps", bufs=1)

            def emit_kvzk(tb, ek, vv):
                for p in range(NPAIR):
                    bank, pp = divmod(p, 4)
                    ekp = ek[:, p * 128:(p + 1) * 128]
                    nc.tensor.matmul(
                        kvps[bank][:, pp * 128:(pp + 1) * 128],
                        ekp, vv[:, p * 128:(p + 1) * 128],
                        start=(tb == 0 and pp == 0),
                        stop=(tb == TB - 1 and pp == 3))
                    nc.tensor.matmul(
                        zkps[:, p:p + 1], ekp, ones,
                        start=(tb == 0 and p == 0),
                        stop=(tb == TB - 1 and p == NPAIR - 1))

            for t in range(NSTART):
                emit_kvzk(t, ek_s[t], vv_s[t])

            # ---- Phase 1 steady state: remaining tbs one at a time ----
            for tb in range(NSTART, TB):
                xtile = xin.tile([128, KC, 128], BF16, tag="x", bufs=6)
                nc.sync.dma_start(xtile, xblk[tb, :, :, :])
                ek = ekv.tile([128, D], BF16, tag="ek", bufs=5)
                vv = ekv.tile([128, D], BF16, tag="v", bufs=5)
                for half in range(2):
                    sl = slice(half * 512, (half + 1) * 512)
                    ps = psum.tile([128, 512], F32, tag="a")
                    for kc in range(KC):
                        nc.tensor.matmul(ps, xtile[:, kc, :], wk_sb[kc][:, sl],
                                         start=(kc == 0), stop=(kc == KC - 1))
                    if with_bias:
                        nc.vector.tensor_add(ps, ps, bk_sb[:, sl])
                    # h1 exp/copy in 128-col pieces: the kv matmuls consume
                    # per-pair columns, so finer ACT ops unblock them sooner.
                    nsp = 1 if half == 0 else 4
                    for i in range(nsp):
                        w = 512 // nsp
                        nc.scalar.activation(
                            ek[:, half * 512 + i * w:half * 512 + (i + 1) * w],
                            ps[:, i * w:(i + 1) * w],
                            mybir.ActivationFunctionType.Exp)
                    ps = psum.tile([128, 512], F32, tag="b")
                    for kc in range(KC):
                        nc.tensor.matmul(ps, xtile[:, kc, :], wv_sb[kc][:, sl],
                                         start=(kc == 0), stop=(kc == KC - 1))
                    if with_bias:
                        nc.vector.tensor_add(ps, ps, bv_sb[:, sl])
                    for i in range(nsp):
                        w = 512 // nsp
                        nc.scalar.copy(
                            vv[:, half * 512 + i * w:half * 512 + (i + 1) * w],
                            ps[:, i * w:(i + 1) * w])
                emit_kvzk(tb, ek, vv)

            # ---- stage compacted partial (kv | Z), AllReduce across seq pair
            # pair p -> cols [64p : 64p+64]; head A rows 0:64, head B 64:128
            stage = accp.tile([128, 520], F32, tag="stage")
            for p in range(NPAIR):
                bank, pp = divmod(p, 4)
                nc.vector.tensor_copy(stage[0:64, 64 * p:64 * p + 64],
                                      kvps[bank][0:64, pp * 128:pp * 128 + 64])
                nc.vector.tensor_copy(stage[64:128, 64 * p:64 * p + 64],
                                      kvps[bank][64:128, pp * 128 + 64:pp * 128 + 128])
            nc.vector.tensor_copy(stage[:, 512:520], zkps[:, 0:8])
            cin = dram.tile([128, 520], F32, tag="cin")
            cout = dram.tile([128, 520], F32, tag="cout")
            nc.sync.dma_start(cin, stage)
            nc.gpsimd.collective_compute(
                "AllReduce", mybir.AluOpType.add,
                replica_groups=[[0, 1], [2, 3], [4, 5], [6, 7]],
                ins=[cin[:].opt()], outs=[cout[:].opt()])
            kvred = accp.tile([128, 520], F32, tag="kvred")
            nc.sync.dma_start(kvred, cout)

            # kv_sb off-diagonal zero-fills: memsets on GpSimd, no inputs, run
            # any time. The 1/Z row scaling happens on ACT after the q sweep.
            kv_sb = [kvsbp.tile([128, 128], BF16, tag=f"kv{p}", name=f"kv{p}")
                     for p in range(NPAIR)]
            for p in range(NPAIR):
                nc.gpsimd.memset(kv_sb[p][0:64, 64:128], 0.0)
                nc.gpsimd.memset(kv_sb[p][64:128, 0:64], 0.0)

            # ---- Phase 2: q projection directly d-major ----
            # eqT[j-block, s] = sum_kc wq[kc, j].T @ xT[kc, s]  (PSUM f32)
            # exp on ACT -> bf16; Z via M8 mask matmul (zq = 8*Z replicated);
            # qt = equ * (1/zq) on DVE -> qtall (the z matmul for block n is
            # emitted after block n+1's projection so the PE never waits on
            # the ACT exp).
            qtall = qtpool.tile([128, NPAIR, T], BF16, tag="qtall")
            pend = []
            zq_n = [0]

            def flush_z(item):
                j, sb, equ = item
                zq = psum.tile([128, 512], F32, tag=("e", "f")[zq_n[0] % 2],
                               bufs=1)
                zq_n[0] += 1
                nc.tensor.matmul(zq, m8, equ, start=True, stop=True)
                rr = qpool.tile([128, 512], F32, tag="rr", bufs=3)
                nc.vector.reciprocal_approx_fast(rr, zq)
                nc.vector.tensor_mul(
                    qtall[:, j, sb * 512:(sb + 1) * 512], equ, rr)

            for sb in range(SB):
                xq = xqp.tile([128, KC, 512], BF16, tag="xq")
                nc.sync.dma_start(xq, xt_v[:, :, sb * 512:(sb + 1) * 512])
                for j in range(NPAIR):
                    ps = psum.tile([128, 512], F32, tag="a")
                    for kc in range(KC):
                        nc.tensor.matmul(
                            ps, wq_sb[kc][:, j * 128:(j + 1) * 128],
                            xq[:, kc, :],
                            start=(kc == 0), stop=(kc == KC - 1))
                    equ = qpool.tile([128, 512], BF16, tag="equ", bufs=3)
                    if with_bias:
                        nc.scalar.activation(equ, ps,
                                             mybir.ActivationFunctionType.Exp,
                                             bias=bq_col[:, j:j + 1])
                    else:
                        nc.scalar.activation(equ, ps,
                                             mybir.ActivationFunctionType.Exp)
                    if pend:
                        flush_z(pend.pop())
                    pend.append((j, sb, equ))
            flush_z(pend.pop())

            # ---- kv 1/Z row scaling (diagonal blocks) on DVE, after the q
            # sweep. (An ACT-side variant via Ln/Exp forces an ACT_TABLE_LOAD
            # that stalls the q-sweep exps ~12us — don't.)
            rz = accp.tile([128, NPAIR], F32, tag="rz")
            nc.vector.reciprocal(rz, kvred[:, 512:520])
            for p in range(NPAIR):
                nc.vector.tensor_scalar_mul(
                    kv_sb[p][0:64, 0:64], kvred[0:64, 64 * p:64 * p + 64],
                    rz[0:64, p:p + 1])
                nc.vector.tensor_scalar_mul(
                    kv_sb[p][64:128, 64:128], kvred[64:128, 64 * p:64 * p + 64],
                    rz[64:128, p:p + 1])

            # ---- attention + output projection, per 512-token block.
            # attn runs one sb ahead of outproj so the PSUM->SBUF copies
            # (split DVE/ACT) complete before outproj consumes them.
            at_tiles = {}

            def emit_attn(sb):
                at = atnp.tile([128, NPAIR, 512], BF16, tag="at")
                at_tiles[sb] = at
                for p in range(NPAIR):
                    aps = psum.tile([128, 512], F32, tag=("c", "d")[p % 2],
                                    bufs=1)
                    nc.tensor.matmul(aps, kv_sb[p],
                                     qtall[:, p, sb * 512:(sb + 1) * 512],
                                     start=True, stop=True)
                    nc.vector.tensor_copy(at[:, p, :], aps)

            def emit_outproj(sb):
                at = at_tiles.pop(sb)
                for tb2 in range(4):
                    for oc in range(2):
                        sl = slice(oc * 512, (oc + 1) * 512)
                        ps = psum.tile([128, 512], F32, tag="b")
                        for kc in range(KC):
                            nc.tensor.matmul(
                                ps, at[:, kc, tb2 * 128:(tb2 + 1) * 128],
                                wp_sb[kc][:, sl],
                                start=(kc == 0), stop=(kc == KC - 1))
                        yt = youtp.tile([128, 512], F32, tag="y")
                        rows = slice((sb * 4 + tb2) * 128,
                                     (sb * 4 + tb2 + 1) * 128)
                        if with_bias:
                            nc.vector.tensor_add(yt, ps, bp_sb[:, sl])
                            nc.sync.dma_start(y[rows, sl], yt)
                        elif sb == SB - 1:
                            # last block: copy+store in 256-col halves so the
                            # tail ACT->DMA chain drains ~1us sooner
                            for hh in range(2):
                                hs = slice(hh * 256, (hh + 1) * 256)
                                nc.scalar.copy(yt[:, hs], ps[:, hs])
                                nc.sync.dma_start(
                                    y[rows, oc * 512 + hh * 256:
                                      oc * 512 + (hh + 1) * 256], yt[:, hs])
                        else:
                            nc.scalar.copy(yt, ps)
                            nc.sync.dma_start(y[rows, sl], yt)

            emit_attn(0)
            for sb in range(SB):
                if sb + 1 < SB:
                    emit_attn(sb + 1)
                emit_outproj(sb)

    nc.compile()
    return nc


_NC = {}


def _get_nc(with_bias=False):
    if with_bias not in _NC:
        _NC[with_bias] = build_program(with_bias=with_bias)
    return _NC[with_bias]


def kernel(x, w_qkv, b_qkv, w_proj, b_proj):
    import ml_dtypes

    bf16 = ml_dtypes.bfloat16
    x = np.asarray(x, dtype=np.float32)
    w_qkv = np.asarray(w_qkv, dtype=np.float32)
    b_qkv = np.asarray(b_qkv, dtype=np.float32)
    w_proj = np.asarray(w_proj, dtype=np.float32)
    b_proj = np.asarray(b_proj, dtype=np.float32)

    bs, seqlen, dim = x.shape
    half = seqlen // 2

    wq = np.ascontiguousarray(w_qkv[0:D].T).astype(bf16)
    wk = np.ascontiguousarray(w_qkv[D:2 * D].T).astype(bf16)
    wv = np.ascontiguousarray(w_qkv[2 * D:3 * D].T).astype(bf16)
    wp = np.ascontiguousarray(w_proj.T).astype(bf16)
    bq, bk, bv = b_qkv[0:D], b_qkv[D:2 * D], b_qkv[2 * D:3 * D]

    # cst: cols 0:128 = 8*block-diag mask (head A rows x head A cols,
    # head B rows x head B cols), col 128 = ones
    m8 = np.zeros((128, 128), dtype=np.float32)
    m8[0:64, 0:64] = 1.0 / SCALE
    m8[64:128, 64:128] = 1.0 / SCALE
    cst = np.concatenate(
        [m8, np.ones((128, 1), dtype=np.float32),
         np.zeros((128, 3), dtype=np.float32)], axis=1).astype(bf16)

    in_maps = []
    for c in range(N_CORES):
        b, s = divmod(c, 2)
        chunk = np.ascontiguousarray(x[b, s * half:(s + 1) * half, :].T).astype(bf16)
        # phase-1 tile-blocked copy: xblk[tb, p, kc, ti] = chunk[kc*128+p, tb*128+ti]
        xb = np.ascontiguousarray(
            chunk.reshape(KC, 128, TB, 128).transpose(2, 1, 0, 3))
        in_maps.append({
            "xt": chunk, "xblk": xb,
            "wq": wq, "wk": wk, "wv": wv, "wp": wp,
            "bq": np.ascontiguousarray(bq), "bk": np.ascontiguousarray(bk),
            "bv": np.ascontiguousarray(bv), "bp": np.ascontiguousarray(b_proj),
            "cst": cst,
        })

    with_bias = bool(np.any(b_qkv)) or bool(np.any(b_proj))
    nc = _get_nc(with_bias)
    global _last_in_maps
    _last_in_maps = in_maps
    res = bass_utils.run_bass_kernel_spmd(nc, in_maps, core_ids=list(range(N_CORES)))

    out = np.empty((bs, seqlen, dim), dtype=np.float32)
    for c in range(N_CORES):
        b, s = divmod(c, 2)
        out[b, s * half:(s + 1) * half, :] = res.results[c]["y"]
    return out


# revision 42
# speedup vs baseline: 1.0505x; 1.0505x over previous
"""Trainium2 Bass kernel for EfficientAttention (linear attention block).

Computation (per batch b, head h):
    qkv = x @ w_qkv.T + b_qkv
    q = softmax(q, axis=head_dim) * head_dim**-0.5
    k = softmax(k, axis=seqlen)
    kv[d,e] = sum_s k[s,d] v[s,e]          (per-head 64x64 state)
    out[s,e] = sum_d q[s,d] kv[d,e]
    y = out @ w_proj.T + b_proj

Sharding: 8 cores = (batch b = c//2, seq half = c%2); 2048 tokens per core,
all 16 heads. The only cross-core coupling is the kv state and the
k-softmax denominator Z (sums over the full 4096 seqlen) -> one small
AllReduce (pairs of cores) of [128, 520] fp32.

v2 design (vs the fp32r baseline):
- All matmul operands in bf16 (1 cycle/row on the PE vs ~2.2 measured for
  fp32 mode; fast weight loads; half the DMA bytes). PSUM stays fp32.
- q projection computed directly d-major (weights stationary, x moving):
  no PE transposes, no per-head ACT normalize sweep. The q-softmax
  denominator comes from one extra matmul per block against a
  block-diagonal 8s mask (folds the 1/sqrt(HD) scale), reciprocal on DVE,
  one elementwise multiply.
- kv state + k-softmax Z accumulate in PSUM banks across the whole token
  loop (per-element has_written accumulation); no DVE adds, and Z lands
  directly in the [128, 8] layout the collective stage wants.
- PE warmup matmuls during the initial weight-DMA wait.
"""

import sys

sys.path.insert(0, "/opt/trn_rl_repo")

import numpy as np

import concourse.bacc as bacc
import concourse.tile as tile
from concourse import mybir
from concourse import bass_utils

F32 = mybir.dt.float32
BF16 = mybir.dt.bfloat16

D = 1024          # model dim (= qkv contraction dim)
T = 2048          # tokens per core (one batch element's half sequence)
NH = 16           # heads
HD = 64           # head dim
NPAIR = 8         # head pairs (2 heads / 128 partitions)
KC = D // 128     # contraction chunks of 128
TB = T // 128     # token blocks of 128
SB = T // 512     # token column blocks of 512
SCALE = HD ** -0.5

N_CORES = 8


def build_program(with_bias=False):
    nc = bacc.Bacc("TRN2", target_bir_lowering=False, num_devices=N_CORES)

    xt = nc.dram_tensor("xt", [D, T], BF16, kind="ExternalInput")      # x chunk, transposed
    xblk = nc.dram_tensor("xblk", [TB, 128, KC, 128], BF16,
                          kind="ExternalInput")  # phase-1 tiles, contiguous
    wq = nc.dram_tensor("wq", [D, D], BF16, kind="ExternalInput")      # w_q.T
    wk = nc.dram_tensor("wk", [D, D], BF16, kind="ExternalInput")      # w_k.T
    wv = nc.dram_tensor("wv", [D, D], BF16, kind="ExternalInput")      # w_v.T
    wp = nc.dram_tensor("wp", [D, D], BF16, kind="ExternalInput")      # w_proj.T
    bq = nc.dram_tensor("bq", [D], F32, kind="ExternalInput")
    bk = nc.dram_tensor("bk", [D], F32, kind="ExternalInput")
    bv = nc.dram_tensor("bv", [D], F32, kind="ExternalInput")
    bp = nc.dram_tensor("bp", [D], F32, kind="ExternalInput")
    cst = nc.dram_tensor("cst", [128, 132], BF16, kind="ExternalInput")  # M8 | ones | pad
    y = nc.dram_tensor("y", [T, D], F32, kind="ExternalOutput")

    xt_v = xt.rearrange("(kc p) t -> p kc t", p=128)
    wq_v = wq.rearrange("(kc p) f -> p kc f", p=128)
    wk_v = wk.rearrange("(kc p) f -> p kc f", p=128)
    wv_v = wv.rearrange("(kc p) f -> p kc f", p=128)
    wp_v = wp.rearrange("(kc p) f -> p kc f", p=128)

    def bias_bcast(b):
        # DRAM [D] broadcast-load to SBUF [128, D] (partition step 0)
        import concourse.bass as bass
        ap = b[:]
        return bass.AP(tensor=ap.tensor, offset=ap.offset, ap=[[0, 128]] + list(ap.ap))

    with tile.TileContext(nc) as tc:
        with (
            tc.tile_pool(name="const", bufs=1) as const,
            tc.tile_pool(name="wpool", bufs=1) as wpool,
            tc.tile_pool(name="xin", bufs=3) as xin,
            tc.tile_pool(name="xq", bufs=2) as xqp,
            tc.tile_pool(name="ekv", bufs=2) as ekv,
            tc.tile_pool(name="acc", bufs=1) as accp,
            tc.tile_pool(name="qpool", bufs=2) as qpool,
            tc.tile_pool(name="qt", bufs=1) as qtpool,
            tc.tile_pool(name="atn", bufs=2) as atnp,
            tc.tile_pool(name="kvsb", bufs=1) as kvsbp,
            tc.tile_pool(name="yout", bufs=3) as youtp,
            tc.tile_pool(name="psum", bufs=2, space="PSUM") as psum,
            tc.tile_pool(name="dram", bufs=1, space="DRAM") as dram,
        ):
            cst_sb = const.tile([128, 132], BF16, tag="cst")
            nc.sync.dma_start(cst_sb, cst[:])
            m8 = cst_sb[:, 0:128]
            ones = cst_sb[:, 128:129]

            if with_bias:
                bk_sb = const.tile([128, D], BF16, tag="bk")
                bv_sb = const.tile([128, D], BF16, tag="bv")
                bp_sb = const.tile([128, D], BF16, tag="bp")
                nc.gpsimd.dma_start(bk_sb, bias_bcast(bk))
                nc.gpsimd.dma_start(bv_sb, bias_bcast(bv))
                nc.gpsimd.dma_start(bp_sb, bias_bcast(bp))
                bq_col = const.tile([128, KC], F32, tag="bqc")
                nc.gpsimd.dma_start(bq_col, bq[:].rearrange("(kc p) -> p kc", p=128))

            # Weights chunked (256KB each) so the k projection can start as
            # soon as early chunks land: wk on the gpsimd ring; wv on the
            # sync ring right behind the first two x tiles; wq/wp trickle
            # in behind wk with phase-1 of slack.
            wkbig = wpool.tile([128, KC, D], BF16, tag="wk", name="wk")
            wvbig = wpool.tile([128, KC, D], BF16, tag="wv", name="wv")
            wqbig = wpool.tile([128, KC, D], BF16, tag="wq", name="wq")
            wpbig = wpool.tile([128, KC, D], BF16, tag="wp", name="wp")
            wk_sb = [wkbig[:, kc, :] for kc in range(KC)]
            wv_sb = [wvbig[:, kc, :] for kc in range(KC)]
            wq_sb = [wqbig[:, kc, :] for kc in range(KC)]
            wp_sb = [wpbig[:, kc, :] for kc in range(KC)]
            NSTART = 4  # tbs processed weight-chunk-major at startup
            for kc in range(KC):
                nc.gpsimd.dma_start(wk_sb[kc], wk_v[:, kc, :])
            xpre = []
            for tb in range(NSTART):
                xtile = xin.tile([128, KC, 128], BF16, tag="x", bufs=6)
                nc.sync.dma_start(xtile, xblk[tb, :, :, :])
                xpre.append(xtile)
            for kc in range(KC):
                nc.sync.dma_start(wv_sb[kc], wv_v[:, kc, :])
            for kc in range(KC):
                nc.gpsimd.dma_start(wq_sb[kc], wq_v[:, kc, :])
                nc.gpsimd.dma_start(wp_sb[kc], wp_v[:, kc, :])

            # PE warmup: dummy matmuls on the (tiny, early-arriving) const
            # tile keep the HAM activity window busy while weights stream in,
            # so real matmuls start at the full 2.4 GHz clock.
            for w in range(24):
                wps = psum.tile([128, 512], F32, tag="a")
                nc.tensor.matmul(wps[:, 0:128], m8, m8, start=True, stop=True)

            # ---- Phase 1 startup: tbs 0..3 processed weight-chunk-major.
            # k projections for 4 tbs x 2 halves fill all 8 PSUM banks, so
            # every arriving wk chunk immediately feeds 8 matmuls (~1.7us of
            # PE work per ~1.4us DMA): the PE streams through the weight
            # window instead of stalling until the full matrix lands.
            tag8 = ("a", "a", "b", "b", "c", "d", "e", "f")

            def startup_proj(w_sb, nm):
                tiles = [psum.tile([128, 512], F32, tag=tag8[i],
                                   bufs=(2 if tag8[i] in ("a", "b") else 1),
                                   name=f"sp_{nm}{i}")
                         for i in range(8)]
                for kc in range(KC):
                    for i in range(8):
                        t, half = divmod(i, 2)
                        nc.tensor.matmul(
                            tiles[i], xpre[t][:, kc, :],
                            w_sb[kc][:, half * 512:(half + 1) * 512],
                            start=(kc == 0), stop=(kc == KC - 1))
                return tiles

            ek_s = [ekv.tile([128, D], BF16, tag="ek", bufs=5, name=f"eks{t}")
                    for t in range(NSTART)]
            vv_s = [ekv.tile([128, D], BF16, tag="v", bufs=5, name=f"vvs{t}")
                    for t in range(NSTART)]
            ktiles = startup_proj(wk_sb, "k")
            for i in range(8):
                t, half = divmod(i, 2)
                sl = slice(half * 512, (half + 1) * 512)
                if with_bias:
                    nc.vector.tensor_add(ktiles[i], ktiles[i], bk_sb[:, sl])
                nc.scalar.activation(ek_s[t][:, sl], ktiles[i],
                                     mybir.ActivationFunctionType.Exp)
            vtiles = startup_proj(wv_sb, "v")
            for i in range(8):
                t, half = divmod(i, 2)
                sl = slice(half * 512, (half + 1) * 512)
                if with_bias:
                    nc.vector.tensor_add(vtiles[i], vtiles[i], bv_sb[:, sl])
                nc.scalar.copy(vv_s[t][:, sl], vtiles[i])

            # kv pairs: bankA = pairs 0-3 (cols 128*p), bankB = pairs 4-7.
            # zk bank: col p = Z partial for pair p's dims [128, 8].
            # start=True only on the first write of each bank (clears the
            # whole bank; the other pairs' first writes land on cleared
            # has_written bits and overwrite).
            kvps = [psum.tile([128, 512], F32, tag=t, name=f"kv{t}", bufs=1)
                    for t in ("c", "d")]
            zkps = psum.tile([128, 512], F32, tag="e", name="zkps", bufs=1)

            def emit_kvzk(tb, ek, vv):
                for p in range(NPAIR):
                    bank, pp = divmod(p, 4)
                    ekp = ek[:, p * 128:(p + 1) * 128]
                    nc.tensor.matmul(
                        kvps[bank][:, pp * 128:(pp + 1) * 128],
                        ekp, vv[:, p * 128:(p + 1) * 128],
                        start=(tb == 0 and pp == 0),
                        stop=(tb == TB - 1 and pp == 3))
                    nc.tensor.matmul(
                        zkps[:, p:p + 1], ekp, ones,
                        start=(tb == 0 and p == 0),
                        stop=(tb == TB - 1 and p == NPAIR - 1))

            for t in range(NSTART):
                emit_kvzk(t, ek_s[t], vv_s[t])

            # ---- Phase 1 steady state: remaining tbs one at a time ----
            for tb in range(NSTART, TB):
                xtile = xin.tile([128, KC, 128], BF16, tag="x", bufs=6)
                nc.sync.dma_start(xtile, xblk[tb, :, :, :])
                ek = ekv.tile([128, D], BF16, tag="ek", bufs=5)
                vv = ekv.tile([128, D], BF16, tag="v", bufs=5)
                for half in range(2):
                    sl = slice(half * 512, (half + 1) * 512)
                    ps = psum.tile([128, 512], F32, tag="a")
                    for kc in range(KC):
                        nc.tensor.matmul(ps, xtile[:, kc, :], wk_sb[kc][:, sl],
                                         start=(kc == 0), stop=(kc == KC - 1))
                    if with_bias:
                        nc.vector.tensor_add(ps, ps, bk_sb[:, sl])
                    # h1 exp/copy in 128-col pieces: the kv matmuls consume
                    # per-pair columns, so finer ACT ops unblock them sooner.
                    nsp = 1 if half == 0 else 4
                    for i in range(nsp):
                        w = 512 // nsp
                        nc.scalar.activation(
                            ek[:, half * 512 + i * w:half * 512 + (i + 1) * w],
                            ps[:, i * w:(i + 1) * w],
                            mybir.ActivationFunctionType.Exp)
                    ps = psum.tile([128, 512], F32, tag="b")
                    for kc in range(KC):
                        nc.tensor.matmul(ps, xtile[:, kc, :], wv_sb[kc][:, sl],
                                         start=(kc == 0), stop=(kc == KC - 1))
                    if with_bias:
                        nc.vector.tensor_add(ps, ps, bv_sb[:, sl])
                    for i in range(nsp):
                        w = 512 // nsp
                        nc.scalar.copy(
                            vv[:, half * 512 + i * w:half * 512 + (i + 1) * w],
                            ps[:, i * w:(i + 1) * w])
                emit_kvzk(tb, ek, vv)

            # ---- stage compacted partial (kv | Z), AllReduce across seq pair
            # pair p -> cols [64p : 64p+64]; head A rows 0:64, head B 64:128
            stage = accp.tile([128, 520], F32, tag="stage")
            for p in range(NPAIR):
                bank, pp = divmod(p, 4)
                nc.vector.tensor_copy(stage[0:64, 64 * p:64 * p + 64],
                                      kvps[bank][0:64, pp * 128:pp * 128 + 64])
                nc.vector.tensor_copy(stage[64:128, 64 * p:64 * p + 64],
                                      kvps[bank][64:128, pp * 128 + 64:pp * 128 + 128])
            nc.vector.tensor_copy(stage[:, 512:520], zkps[:, 0:8])
            cin = dram.tile([128, 520], F32, tag="cin")
            cout = dram.tile([128, 520], F32, tag="cout")
            nc.sync.dma_start(cin, stage)
            nc.gpsimd.collective_compute(
                "AllReduce", mybir.AluOpType.add,
                replica_groups=[[0, 1], [2, 3], [4, 5], [6, 7]],
                ins=[cin[:].opt()], outs=[cout[:].opt()])
            kvred = accp.tile([128, 520], F32, tag="kvred")
            nc.sync.dma_start(kvred, cout)

            # kv_sb off-diagonal zero-fills: memsets on GpSimd, no inputs, run
            # any time. The 1/Z row scaling happens on ACT after the q sweep.
            kv_sb = [kvsbp.tile([128, 128], BF16, tag=f"kv{p}", name=f"kv{p}")
                     for p in range(NPAIR)]
            for p in range(NPAIR):
                nc.gpsimd.memset(kv_sb[p][0:64, 64:128], 0.0)
                nc.gpsimd.memset(kv_sb[p][64:128, 0:64], 0.0)

            # ---- Phase 2: q projection directly d-major ----
            # eqT[j-block, s] = sum_kc wq[kc, j].T @ xT[kc, s]  (PSUM f32)
            # exp on ACT -> bf16; Z via M8 mask matmul (zq = 8*Z replicated);
            # qt = equ * (1/zq) on DVE -> qtall (the z matmul for block n is
            # emitted after block n+1's projection so the PE never waits on
            # the ACT exp).
            qtall = qtpool.tile([128, NPAIR, T], BF16, tag="qtall")
            pend = []
            zq_n = [0]

            def flush_z(item):
                j, sb, equ = item
                zq = psum.tile([128, 512], F32, tag=("e", "f")[zq_n[0] % 2],
                               bufs=1)
                zq_n[0] += 1
                nc.tensor.matmul(zq, m8, equ, start=True, stop=True)
                rr = qpool.tile([128, 512], F32, tag="rr", bufs=3)
                nc.vector.reciprocal_approx_fast(rr, zq)
                nc.vector.tensor_mul(
                    qtall[:, j, sb * 512:(sb + 1) * 512], equ, rr)

            for sb in range(SB):
                xq = xqp.tile([128, KC, 512], BF16, tag="xq")
                nc.sync.dma_start(xq, xt_v[:, :, sb * 512:(sb + 1) * 512])
                for j in range(NPAIR):
                    ps = psum.tile([128, 512], F32, tag="a")
                    for kc in range(KC):
                        nc.tensor.matmul(
                            ps, wq_sb[kc][:, j * 128:(j + 1) * 128],
                            xq[:, kc, :],
                            start=(kc == 0), stop=(kc == KC - 1))
                    equ = qpool.tile([128, 512], BF16, tag="equ", bufs=3)
                    if with_bias:
                        nc.scalar.activation(equ, ps,
                                             mybir.ActivationFunctionType.Exp,
                                             bias=bq_col[:, j:j + 1])
                    else:
                        nc.scalar.activation(equ, ps,
                                             mybir.ActivationFunctionType.Exp)
                    if pend:
                        flush_z(pend.pop())
                    pend.append((j, sb, equ))
            flush_z(pend.pop())

            # ---- kv 1/Z row scaling (diagonal blocks) on DVE, after the q
            # sweep. (An ACT-side variant via Ln/Exp forces an ACT_TABLE_LOAD
            # that stalls the q-sweep exps ~12us — don't.)
            rz = accp.tile([128, NPAIR], F32, tag="rz")
            nc.vector.reciprocal(rz, kvred[:, 512:520])
            for p in range(NPAIR):
                nc.vector.tensor_scalar_mul(
                    kv_sb[p][0:64, 0:64], kvred[0:64, 64 * p:64 * p + 64],
                    rz[0:64, p:p + 1])
                nc.vector.tensor_scalar_mul(
                    kv_sb[p][64:128, 64:128], kvred[64:128, 64 * p:64 * p + 64],
                    rz[64:128, p:p + 1])

            # ---- attention + output projection, per 512-token block.
            # attn runs one sb ahead of outproj so the PSUM->SBUF copies
            # (split DVE/ACT) complete before outproj consumes them.
            at_tiles = {}

            def emit_attn(sb):
                at = atnp.tile([128, NPAIR, 512], BF16, tag="at")
                at_tiles[sb] = at
                for p in range(NPAIR):
                    aps = psum.tile([128, 512], F32, tag=("c", "d")[p % 2],
                                    bufs=1)
                    nc.tensor.matmul(aps, kv_sb[p],
                                     qtall[:, p, sb * 512:(sb + 1) * 512],
                                     start=True, stop=True)
                    nc.vector.tensor_copy(at[:, p, :], aps)

            def emit_outproj(sb):
                at = at_tiles.pop(sb)
                for tb2 in range(4):
                    for oc in range(2):
                        sl = slice(oc * 512, (oc + 1) * 512)
                        ps = psum.tile([128, 512], F32, tag="b")
                        for kc in range(KC):
                            nc.tensor.matmul(
                                ps, at[:, kc, tb2 * 128:(tb2 + 1) * 128],
                                wp_sb[kc][:, sl],
                                start=(kc == 0), stop=(kc == KC - 1))
                        yt = youtp.tile([128, 512], F32, tag="y")
                        if with_bias:
                            nc.vector.tensor_add(yt, ps, bp_sb[:, sl])
                        else:
                            nc.scalar.copy(yt, ps)
                        nc.sync.dma_start(
                            y[(sb * 4 + tb2) * 128:(sb * 4 + tb2 + 1) * 128, sl],
                            yt)

            emit_attn(0)
            for sb in range(SB):
                if sb + 1 < SB:
                    emit_attn(sb + 1)
                emit_outproj(sb)

    nc.compile()
    return nc


_NC = {}


def _get_nc(with_bias=False):
    if with_bias not in _NC:
        _NC[with_bias] = build_program(with_bias=with_bias)
    return _NC[with_bias]


def kernel(x, w_qkv, b_qkv, w_proj, b_proj):
    import ml_dtypes

    bf16 = ml_dtypes.bfloat16
    x = np.asarray(x, dtype=np.float32)
    w_qkv = np.asarray(w_qkv, dtype=np.float32)
    b_qkv = np.asarray(b_qkv, dtype=np.float32)
    w_proj = np.asarray(w_proj, dtype=np.float32)
    b_proj = np.asarray(b_proj, dtype=np.float32)

    bs, seqlen, dim = x.shape
    half = seqlen // 2

    wq = np.ascontiguousarray(w_qkv[0:D].T).astype(bf16)
    wk = np.ascontiguousarray(w_qkv[D:2 * D].T).astype(bf16)
    wv = np.ascontiguousarray(w_qkv[2 * D:3 * D].T).astype(bf16)
    wp = np.ascontiguousarray(w_proj.T).astype(bf16)
    bq, bk, bv = b_qkv[0:D], b_qkv[D:2 * D], b_qkv[2 * D:3 * D]

    # cst: cols 0:128 = 8*block-diag mask (head A rows x head A cols,
    # head B rows x head B cols), col 128 = ones
    m8 = np.zeros((128, 128), dtype=np.float32)
    m8[0:64, 0:64] = 1.0 / SCALE
    m8[64:128, 64:128] = 1.0 / SCALE
    cst = np.concatenate(
        [m8, np.ones((128, 1), dtype=np.float32),
         np.zeros((128, 3), dtype=np.float32)], axis=1).astype(bf16)

    in_maps = []
    for c in range(N_CORES):
        b, s = divmod(c, 2)
        chunk = np.ascontiguousarray(x[b, s * half:(s + 1) * half, :].T).astype(bf16)
        # phase-1 tile-blocked copy: xblk[tb, p, kc, ti] = chunk[kc*128+p, tb*128+ti]
        xb = np.ascontiguousarray(
            chunk.reshape(KC, 128, TB, 128).transpose(2, 1, 0, 3))
        in_maps.append({
            "xt": chunk, "xblk": xb,
            "wq": wq, "wk": wk, "wv": wv, "wp": wp,
            "bq": np.ascontiguousarray(bq), "bk": np.ascontiguousarray(bk),
            "bv": np.ascontiguousarray(bv), "bp": np.ascontiguousarray(b_proj),
            "cst": cst,
        })

    with_bias = bool(np.any(b_qkv)) or bool(np.any(b_proj))
    nc = _get_nc(with_bias)
    global _last_in_maps
    _last_in_maps = in_maps
    res = bass_utils.run_bass_kernel_spmd(nc, in_maps, core_ids=list(range(N_CORES)))

    out = np.empty((bs, seqlen, dim), dtype=np.float32)
    for c in range(N_CORES):
        b, s = divmod(c, 2)
        out[b, s * half:(s + 1) * half, :] = res.results[c]["y"]
    return out
